# revision 2
# baseline (speedup 1.0000x reference)
"""Trainium2 Bass kernel for nn_ActorNetwork (moe_routing).

Design (host-routed expert parallelism, zero collectives):
  reference semantics: with perm = stable argsort(idx),
    h_f[i] = relu(relu(state[perm[i]] @ W1[g(i)] + b1[g(i)]) @ W2 + b2)
    out[i] = tanh(h_f[i] @ W3[idx[i]] + b3[idx[i]])
  where g(i) (the W1 expert of sorted-position i) depends only on which
  sorted-count block position i falls into.  Core c takes exactly the sorted
  block of game c -> its layer-1 is ONE dense matmul with only W1[c].  Within
  the core, rows are sub-grouped by head game idx[i] so layer-3 is 8 dense
  per-group matmuls.  All routing (gather of state rows in, scatter of output
  rows back) happens on the host during shard/unshard.  Groups are stored
  sorted by size (descending) so the SPMD-uniform slot capacities
  M_j = max_core(j-th largest group) give ~1% padding.

  On-device layout is feature-major: activations live as [feature, row] so
  every matmul is lhsT=weight-tile [K=128, M=128], rhs=activation [K=128,
  N=rows], PSUM out [M features, rows].  Compute dtype bf16, f32 PSUM.
"""

import numpy as np
import ml_dtypes

_BF16 = ml_dtypes.bfloat16
_NCORES = 8
_graph_cache: dict = {}


def _make_plan(idx: np.ndarray, G: int):
    """Host routing plan: which (sorted-position) rows go to which core/slot."""
    idx = np.asarray(idx)
    perm = np.argsort(idx, kind="stable")
    counts = np.bincount(idx, minlength=G)
    cum = np.zeros(G + 1, dtype=np.int64)
    cum[1:] = np.cumsum(counts)

    core_groups = []  # per core: list of (head_game, sorted_positions) desc by size
    for c in range(G):
        pos = np.arange(cum[c], cum[c + 1])
        heads = idx[pos]
        groups = [(b, pos[heads == b]) for b in range(G)]
        groups.sort(key=lambda t: (-len(t[1]), t[0]))
        core_groups.append(groups)

    sizes = np.array([[len(p) for _, p in groups] for groups in core_groups])
    M = sizes.max(axis=0)          # slot capacity per position (SPMD-uniform)
    keep = M > 0
    M = M[keep]
    core_groups = [[g for g, k in zip(groups, keep) if k] for groups in core_groups]
    NG = len(M)
    starts = np.zeros(NG + 1, dtype=np.int64)
    starts[1:] = np.cumsum(M)
    N = int(starts[-1])
    return perm, core_groups, M, starts, N


def _build_graph(D, H1, H2, A, NG, starts, N):
    """Build + finalize the SPMD Bass graph (identical for all cores)."""
    from concourse import bacc
    import concourse.mybir as mybir
    from concourse.tile import TileContext

    bf = mybir.dt.bfloat16
    f32 = mybir.dt.float32
    KD, K1, K2 = D // 128, H1 // 128, H2 // 128
    M1, M2 = H1 // 128, H2 // 128
    assert D % 128 == 0 and H1 % 128 == 0 and H2 % 128 == 0 and A <= 128

    chunks = []
    c0 = 0
    while c0 < N:
        cw = min(512, N - c0)
        chunks.append((c0, cw))
        c0 += cw

    nc = bacc.Bacc("TRN2")
    st_ext = nc.declare_dram_parameter("st", [KD, 128, N], bf, isOutput=False)
    w1_ext = nc.declare_dram_parameter("w1", [KD, 128, H1], bf, isOutput=False)
    w2_ext = nc.declare_dram_parameter("w2", [K1, 128, H2], bf, isOutput=False)
    w3_ext = nc.declare_dram_parameter("w3", [NG, 128, K2 * A], bf, isOutput=False)
    b1_ext = nc.declare_dram_parameter("b1s", [128, M1], f32, isOutput=False)
    b2_ext = nc.declare_dram_parameter("b2s", [128, M2], f32, isOutput=False)
    b3_ext = nc.declare_dram_parameter("b3s", [A, NG], f32, isOutput=False)
    out_ext = nc.declare_dram_parameter("out", [A, N], f32, isOutput=True)

    add = mybir.AluOpType.add
    amax = mybir.AluOpType.max
    Tanh = mybir.ActivationFunctionType.Tanh

    with TileContext(nc) as tc:
        with (
            tc.tile_pool(name="weights", bufs=1) as wp,
            tc.tile_pool(name="acts", bufs=1) as ap,
            tc.tile_pool(name="psum", bufs=4, space="PSUM") as pp,
            tc.tile_pool(name="psum3", bufs=2, space="PSUM") as pp3,
        ):
            st = [ap.tile([128, N], bf, name=f"st{k}", tag=f"st{k}") for k in range(KD)]
            w1 = [wp.tile([128, H1], bf, name=f"w1_{k}", tag=f"w1_{k}") for k in range(KD)]
            w2 = [wp.tile([128, H2], bf, name=f"w2_{k}", tag=f"w2_{k}") for k in range(K1)]
            w3 = [wp.tile([128, K2 * A], bf, name=f"w3_{j}", tag=f"w3_{j}") for j in range(NG)]
            b1t = wp.tile([128, M1], f32, name="b1t", tag="b1t")
            b2t = wp.tile([128, M2], f32, name="b2t", tag="b2t")
            b3t = wp.tile([A, NG], f32, name="b3t", tag="b3t")
            h1 = [ap.tile([128, N], bf, name=f"h1_{m}", tag=f"h1_{m}") for m in range(M1)]
            hf = [ap.tile([128, N], bf, name=f"hf_{m}", tag=f"hf_{m}") for m in range(M2)]
            osb = ap.tile([A, N], f32, name="osb", tag="osb")

            for k in range(KD):
                nc.sync.dma_start(w1[k][:], w1_ext[k])
                nc.sync.dma_start(st[k][:], st_ext[k])
            for k in range(K1):
                nc.sync.dma_start(w2[k][:], w2_ext[k])
            for j in range(NG):
                nc.sync.dma_start(w3[j][:], w3_ext[j])
            nc.sync.dma_start(b1t[:], b1_ext[:])
            nc.sync.dma_start(b2t[:], b2_ext[:])
            nc.sync.dma_start(b3t[:], b3_ext[:])

            # L3 group j is ready once chunks covering [starts[j], starts[j+1])
            # are done; emit it eagerly after that chunk for scheduling overlap.
            done_j = 0

            for ci, (c0, cw) in enumerate(chunks):
                sl = slice(c0, c0 + cw)
                # L1: h1[m][:, sl] = relu(W1[:, m-tile].T @ st[:, sl] + b1)
                for m in range(M1):
                    ps = pp.tile([128, cw], f32, name="ps", tag="ps")
                    for k in range(KD):
                        nc.tensor.matmul(
                            ps[:],
                            w1[k][:, m * 128 : (m + 1) * 128],
                            st[k][:, sl],
                            start=(k == 0),
                            stop=(k == KD - 1),
                        )
                    nc.any.tensor_scalar(
                        h1[m][:, sl], ps[:], b1t[:, m : m + 1], 0.0, add, amax
                    )
                # L2
                for m in range(M2):
                    ps = pp.tile([128, cw], f32, name="ps", tag="ps")
                    for k in range(K1):
                        nc.tensor.matmul(
                            ps[:],
                            w2[k][:, m * 128 : (m + 1) * 128],
                            h1[k][:, sl],
                            start=(k == 0),
                            stop=(k == K1 - 1),
                        )
                    nc.any.tensor_scalar(
                        hf[m][:, sl], ps[:], b2t[:, m : m + 1], 0.0, add, amax
                    )
                # L3 for fully-covered groups
                lim = c0 + cw
                while done_j < NG and starts[done_j + 1] <= lim:
                    j = done_j
                    sj, ej = int(starts[j]), int(starts[j + 1])
                    mj = ej - sj
                    ps = pp3.tile([A, mj], f32, name="ps3", tag="ps3")
                    for k in range(K2):
                        nc.tensor.matmul(
                            ps[:],
                            w3[j][:, k * A : (k + 1) * A],
                            hf[k][:, sj:ej],
                            start=(k == 0),
                            stop=(k == K2 - 1),
                        )
                    nc.scalar.activation(
                        osb[:, sj:ej], ps[:], Tanh, bias=b3t[:, j : j + 1]
                    )
                    nc.sync.dma_start(out_ext[:, sj:ej], osb[:, sj:ej])
                    done_j += 1
            assert done_j == NG

    nc.finalize()
    return nc


def _prepare(state, idx, W1, b1, W2, b2, W3, b3):
    state = np.ascontiguousarray(np.asarray(state, dtype=np.float32))
    idx = np.asarray(idx)
    W1 = np.asarray(W1, dtype=np.float32)
    b1 = np.asarray(b1, dtype=np.float32)
    W2 = np.asarray(W2, dtype=np.float32)
    b2 = np.asarray(b2, dtype=np.float32)
    W3 = np.asarray(W3, dtype=np.float32)
    b3 = np.asarray(b3, dtype=np.float32)

    B, D = state.shape
    G, _, H1 = W1.shape
    H2 = W2.shape[1]
    A = W3.shape[2]
    KD, K1, K2 = D // 128, H1 // 128, H2 // 128

    perm, core_groups, M, starts, N = _make_plan(idx, G)
    NG = len(M)

    key = (D, H1, H2, A, NG, tuple(int(x) for x in starts), N)
    if key not in _graph_cache:
        _graph_cache[key] = _build_graph(D, H1, H2, A, NG, starts, N)
    nc = _graph_cache[key]

    w2_h = np.ascontiguousarray(W2.astype(_BF16).reshape(K1, 128, H2))
    b2_h = np.ascontiguousarray(b2.reshape(K1, 128).T.astype(np.float32))

    in_maps = []
    scatters = []  # per core: list of (sorted_positions, col_start)
    for c in range(G):
        sT = np.zeros((D, N), dtype=_BF16)
        w3_h = np.zeros((NG, 128, K2 * A), dtype=_BF16)
        b3_h = np.zeros((A, NG), dtype=np.float32)
        sc = []
        for j, (head, pos) in enumerate(core_groups[c]):
            s0 = int(starts[j])
            if len(pos):
                sT[:, s0 : s0 + len(pos)] = state[perm[pos]].T.astype(_BF16)
                sc.append((pos, s0))
            # [ki, ko, a] layout so each SBUF k-slice [:, k*A:(k+1)*A] is [128, A]
            w3_h[j] = (
                W3[head].astype(_BF16).reshape(K2, 128, A).transpose(1, 0, 2).reshape(128, K2 * A)
            )
            b3_h[:, j] = b3[head]
        in_maps.append(
            {
                "st": np.ascontiguousarray(sT.reshape(KD, 128, N)),
                "w1": np.ascontiguousarray(W1[c].astype(_BF16).reshape(KD, 128, H1)),
                "w2": w2_h,
                "w3": np.ascontiguousarray(w3_h),
                "b1s": np.ascontiguousarray(b1[c].reshape(K1, 128).T.astype(np.float32)),
                "b2s": b2_h,
                "b3s": b3_h,
            }
        )
        scatters.append(sc)
    return nc, in_maps, scatters, (B, A)


def _run(state, idx, W1, b1, W2, b2, W3, b3, trace=False, trace_kwargs=None):
    from concourse.bass_utils import run_bass_kernel_spmd

    nc, in_maps, scatters, (B, A) = _prepare(state, idx, W1, b1, W2, b2, W3, b3)
    res = run_bass_kernel_spmd(
        nc,
        in_maps,
        core_ids=list(range(_NCORES)),
        trace=trace,
        **(trace_kwargs or {}),
    )
    out = np.zeros((B, A), dtype=np.float32)
    for c in range(len(scatters)):
        o = np.asarray(res.results[c]["out"], dtype=np.float32)  # [A, N]
        for pos, s0 in scatters[c]:
            out[pos] = o[:, s0 : s0 + len(pos)].T
    return out, res


def kernel(**inputs) -> np.ndarray:
    out, _ = _run(**inputs)
    return out


# revision 3
# speedup vs baseline: 1.1539x; 1.1539x over previous
"""Trainium2 Bass kernel for nn_ActorNetwork (moe_routing).

Design (host-routed expert parallelism, zero collectives):
  reference semantics: with perm = stable argsort(idx),
    h_f[i] = relu(relu(state[perm[i]] @ W1[g(i)] + b1[g(i)]) @ W2 + b2)
    out[i] = tanh(h_f[i] @ W3[idx[i]] + b3[idx[i]])
  where g(i) (the W1 expert of sorted-position i) depends only on which
  sorted-count block position i falls into.  Core c takes exactly the sorted
  block of game c -> its layer-1 is ONE dense matmul with only W1[c].  Within
  the core, rows are sub-grouped by head game idx[i] so layer-3 is 8 dense
  per-group matmuls.  All routing (gather of state rows in, scatter of output
  rows back) happens on the host during shard/unshard.  Groups are stored
  sorted by size (descending) so the SPMD-uniform slot capacities
  M_j = max_core(j-th largest group) give ~1% padding.

  On-device layout is feature-major: activations live as [feature, row] so
  every matmul is lhsT=weight-tile [K=128, M=128], rhs=activation [K=128,
  N=rows], PSUM out [M features, rows].  Compute dtype bf16, f32 PSUM.
"""

import numpy as np
import ml_dtypes

_BF16 = ml_dtypes.bfloat16
_NCORES = 8
_graph_cache: dict = {}


def _make_plan(idx: np.ndarray, G: int):
    """Host routing plan: which (sorted-position) rows go to which core/slot."""
    idx = np.asarray(idx)
    perm = np.argsort(idx, kind="stable")
    counts = np.bincount(idx, minlength=G)
    cum = np.zeros(G + 1, dtype=np.int64)
    cum[1:] = np.cumsum(counts)

    core_groups = []  # per core: list of (head_game, sorted_positions) desc by size
    for c in range(G):
        pos = np.arange(cum[c], cum[c + 1])
        heads = idx[pos]
        groups = [(b, pos[heads == b]) for b in range(G)]
        groups.sort(key=lambda t: (-len(t[1]), t[0]))
        core_groups.append(groups)

    sizes = np.array([[len(p) for _, p in groups] for groups in core_groups])
    M = sizes.max(axis=0)          # slot capacity per position (SPMD-uniform)
    keep = M > 0
    M = M[keep]
    core_groups = [[g for g, k in zip(groups, keep) if k] for groups in core_groups]
    NG = len(M)
    starts = np.zeros(NG + 1, dtype=np.int64)
    starts[1:] = np.cumsum(M)
    N = int(starts[-1])
    return perm, core_groups, M, starts, N


def _build_graph(D, H1, H2, A, NG, starts, N):
    """Build + finalize the SPMD Bass graph (identical for all cores)."""
    from concourse import bacc
    import concourse.mybir as mybir
    from concourse.tile import TileContext

    bf = mybir.dt.bfloat16
    f32 = mybir.dt.float32
    KD, K1, K2 = D // 128, H1 // 128, H2 // 128
    M1, M2 = H1 // 128, H2 // 128
    assert D % 128 == 0 and H1 % 128 == 0 and H2 % 128 == 0 and A <= 128

    chunks = []
    c0 = 0
    while c0 < N:
        cw = min(512, N - c0)
        chunks.append((c0, cw))
        c0 += cw

    nc = bacc.Bacc("TRN2")
    st_ext = nc.declare_dram_parameter("st", [KD, 128, N], bf, isOutput=False)
    w1_ext = nc.declare_dram_parameter("w1", [KD, 128, H1], bf, isOutput=False)
    w2_ext = nc.declare_dram_parameter("w2", [K1, 128, H2], bf, isOutput=False)
    w3_ext = nc.declare_dram_parameter("w3", [NG, 128, K2 * A], bf, isOutput=False)
    b1_ext = nc.declare_dram_parameter("b1s", [128, M1], f32, isOutput=False)
    b2_ext = nc.declare_dram_parameter("b2s", [128, M2], f32, isOutput=False)
    b3_ext = nc.declare_dram_parameter("b3s", [A, NG], f32, isOutput=False)
    out_ext = nc.declare_dram_parameter("out", [A, N], f32, isOutput=True)

    add = mybir.AluOpType.add
    amax = mybir.AluOpType.max
    Tanh = mybir.ActivationFunctionType.Tanh

    with TileContext(nc) as tc:
        with (
            tc.tile_pool(name="weights", bufs=1) as wp,
            tc.tile_pool(name="acts", bufs=1) as ap,
            tc.tile_pool(name="psum", bufs=4, space="PSUM") as pp,
            tc.tile_pool(name="psum3", bufs=2, space="PSUM") as pp3,
        ):
            st = [ap.tile([128, N], bf, name=f"st{k}", tag=f"st{k}") for k in range(KD)]
            w1 = [wp.tile([128, H1], bf, name=f"w1_{k}", tag=f"w1_{k}") for k in range(KD)]
            w2 = [wp.tile([128, H2], bf, name=f"w2_{k}", tag=f"w2_{k}") for k in range(K1)]
            w3 = [wp.tile([128, K2 * A], bf, name=f"w3_{j}", tag=f"w3_{j}") for j in range(NG)]
            b1t = wp.tile([128, M1], f32, name="b1t", tag="b1t")
            b2t = wp.tile([128, M2], f32, name="b2t", tag="b2t")
            b3t = wp.tile([A, NG], f32, name="b3t", tag="b3t")
            h1 = [ap.tile([128, N], bf, name=f"h1_{m}", tag=f"h1_{m}") for m in range(M1)]
            hf = [ap.tile([128, N], bf, name=f"hf_{m}", tag=f"hf_{m}") for m in range(M2)]
            osb = ap.tile([A, N], f32, name="osb", tag="osb")

            for k in range(KD):
                nc.sync.dma_start(w1[k][:], w1_ext[k])
                nc.sync.dma_start(st[k][:], st_ext[k])
            for k in range(K1):
                nc.sync.dma_start(w2[k][:], w2_ext[k])
            for j in range(NG):
                nc.sync.dma_start(w3[j][:], w3_ext[j])
            nc.sync.dma_start(b1t[:], b1_ext[:])
            nc.sync.dma_start(b2t[:], b2_ext[:])
            nc.sync.dma_start(b3t[:], b3_ext[:])

            # L3 group j is ready once chunks covering [starts[j], starts[j+1])
            # are done; emit it eagerly after that chunk for scheduling overlap.
            done_j = 0

            for ci, (c0, cw) in enumerate(chunks):
                sl = slice(c0, c0 + cw)
                # L1: h1[m][:, sl] = relu(W1[:, m-tile].T @ st[:, sl] + b1)
                for m in range(M1):
                    ps = pp.tile([128, cw], f32, name="ps", tag="ps")
                    for k in range(KD):
                        nc.tensor.matmul(
                            ps[:],
                            w1[k][:, m * 128 : (m + 1) * 128],
                            st[k][:, sl],
                            start=(k == 0),
                            stop=(k == KD - 1),
                        )
                    nc.vector.tensor_scalar(
                        h1[m][:, sl], ps[:], b1t[:, m : m + 1], 0.0, add, amax
                    )
                # L2
                for m in range(M2):
                    ps = pp.tile([128, cw], f32, name="ps", tag="ps")
                    for k in range(K1):
                        nc.tensor.matmul(
                            ps[:],
                            w2[k][:, m * 128 : (m + 1) * 128],
                            h1[k][:, sl],
                            start=(k == 0),
                            stop=(k == K1 - 1),
                        )
                    nc.vector.tensor_scalar(
                        hf[m][:, sl], ps[:], b2t[:, m : m + 1], 0.0, add, amax
                    )
                # L3 for fully-covered groups
                lim = c0 + cw
                while done_j < NG and starts[done_j + 1] <= lim:
                    j = done_j
                    sj, ej = int(starts[j]), int(starts[j + 1])
                    mj = ej - sj
                    ps = pp3.tile([A, mj], f32, name="ps3", tag="ps3")
                    for k in range(K2):
                        nc.tensor.matmul(
                            ps[:],
                            w3[j][:, k * A : (k + 1) * A],
                            hf[k][:, sj:ej],
                            start=(k == 0),
                            stop=(k == K2 - 1),
                        )
                    nc.scalar.activation(
                        osb[:, sj:ej], ps[:], Tanh, bias=b3t[:, j : j + 1]
                    )
                    nc.sync.dma_start(out_ext[:, sj:ej], osb[:, sj:ej])
                    done_j += 1
            assert done_j == NG

    nc.finalize()
    return nc


def _prepare(state, idx, W1, b1, W2, b2, W3, b3):
    state = np.ascontiguousarray(np.asarray(state, dtype=np.float32))
    idx = np.asarray(idx)
    W1 = np.asarray(W1, dtype=np.float32)
    b1 = np.asarray(b1, dtype=np.float32)
    W2 = np.asarray(W2, dtype=np.float32)
    b2 = np.asarray(b2, dtype=np.float32)
    W3 = np.asarray(W3, dtype=np.float32)
    b3 = np.asarray(b3, dtype=np.float32)

    B, D = state.shape
    G, _, H1 = W1.shape
    H2 = W2.shape[1]
    A = W3.shape[2]
    KD, K1, K2 = D // 128, H1 // 128, H2 // 128

    perm, core_groups, M, starts, N = _make_plan(idx, G)
    NG = len(M)

    key = (D, H1, H2, A, NG, tuple(int(x) for x in starts), N)
    if key not in _graph_cache:
        _graph_cache[key] = _build_graph(D, H1, H2, A, NG, starts, N)
    nc = _graph_cache[key]

    w2_h = np.ascontiguousarray(W2.astype(_BF16).reshape(K1, 128, H2))
    b2_h = np.ascontiguousarray(b2.reshape(K1, 128).T.astype(np.float32))

    in_maps = []
    scatters = []  # per core: list of (sorted_positions, col_start)
    for c in range(G):
        sT = np.zeros((D, N), dtype=_BF16)
        w3_h = np.zeros((NG, 128, K2 * A), dtype=_BF16)
        b3_h = np.zeros((A, NG), dtype=np.float32)
        sc = []
        for j, (head, pos) in enumerate(core_groups[c]):
            s0 = int(starts[j])
            if len(pos):
                sT[:, s0 : s0 + len(pos)] = state[perm[pos]].T.astype(_BF16)
                sc.append((pos, s0))
            # [ki, ko, a] layout so each SBUF k-slice [:, k*A:(k+1)*A] is [128, A]
            w3_h[j] = (
                W3[head].astype(_BF16).reshape(K2, 128, A).transpose(1, 0, 2).reshape(128, K2 * A)
            )
            b3_h[:, j] = b3[head]
        in_maps.append(
            {
                "st": np.ascontiguousarray(sT.reshape(KD, 128, N)),
                "w1": np.ascontiguousarray(W1[c].astype(_BF16).reshape(KD, 128, H1)),
                "w2": w2_h,
                "w3": np.ascontiguousarray(w3_h),
                "b1s": np.ascontiguousarray(b1[c].reshape(K1, 128).T.astype(np.float32)),
                "b2s": b2_h,
                "b3s": b3_h,
            }
        )
        scatters.append(sc)
    return nc, in_maps, scatters, (B, A)


def _run(state, idx, W1, b1, W2, b2, W3, b3, trace=False, trace_kwargs=None):
    from concourse.bass_utils import run_bass_kernel_spmd

    nc, in_maps, scatters, (B, A) = _prepare(state, idx, W1, b1, W2, b2, W3, b3)
    res = run_bass_kernel_spmd(
        nc,
        in_maps,
        core_ids=list(range(_NCORES)),
        trace=trace,
        **(trace_kwargs or {}),
    )
    out = np.zeros((B, A), dtype=np.float32)
    for c in range(len(scatters)):
        o = np.asarray(res.results[c]["out"], dtype=np.float32)  # [A, N]
        for pos, s0 in scatters[c]:
            out[pos] = o[:, s0 : s0 + len(pos)].T
    return out, res


def kernel(**inputs) -> np.ndarray:
    out, _ = _run(**inputs)
    return out


# revision 4
# speedup vs baseline: 1.3289x; 1.1516x over previous
"""Trainium2 Bass kernel for nn_ActorNetwork (moe_routing).

Design (host-routed expert parallelism, zero collectives):
  reference semantics: with perm = stable argsort(idx),
    h_f[i] = relu(relu(state[perm[i]] @ W1[g(i)] + b1[g(i)]) @ W2 + b2)
    out[i] = tanh(h_f[i] @ W3[idx[i]] + b3[idx[i]])
  where g(i) (the W1 expert of sorted-position i) depends only on which
  sorted-count block position i falls into.  Core c takes exactly the sorted
  block of game c -> its layer-1 is ONE dense matmul with only W1[c].  Within
  the core, rows are sub-grouped by head game idx[i] so layer-3 is 8 dense
  per-group matmuls.  All routing (gather of state rows in, scatter of output
  rows back) happens on the host during shard/unshard.  Groups are stored
  sorted by size (descending) so the SPMD-uniform slot capacities
  M_j = max_core(j-th largest group) give ~1% padding.

  On-device layout is feature-major: activations live as [feature, row] so
  every matmul is lhsT=weight-tile [K=128, M=128], rhs=activation [K=128,
  N=rows], PSUM out [M features, rows].  Compute dtype bf16, f32 PSUM.
"""

import numpy as np
import ml_dtypes

_BF16 = ml_dtypes.bfloat16
_NCORES = 8
_graph_cache: dict = {}


def _make_plan(idx: np.ndarray, G: int):
    """Host routing plan: which (sorted-position) rows go to which core/slot."""
    idx = np.asarray(idx)
    perm = np.argsort(idx, kind="stable")
    counts = np.bincount(idx, minlength=G)
    cum = np.zeros(G + 1, dtype=np.int64)
    cum[1:] = np.cumsum(counts)

    core_groups = []  # per core: list of (head_game, sorted_positions) desc by size
    for c in range(G):
        pos = np.arange(cum[c], cum[c + 1])
        heads = idx[pos]
        groups = [(b, pos[heads == b]) for b in range(G)]
        groups.sort(key=lambda t: (-len(t[1]), t[0]))
        core_groups.append(groups)

    sizes = np.array([[len(p) for _, p in groups] for groups in core_groups])
    M = sizes.max(axis=0)          # slot capacity per position (SPMD-uniform)
    keep = M > 0
    M = M[keep]
    core_groups = [[g for g, k in zip(groups, keep) if k] for groups in core_groups]
    NG = len(M)
    starts = np.zeros(NG + 1, dtype=np.int64)
    starts[1:] = np.cumsum(M)
    N = int(starts[-1])
    return perm, core_groups, M, starts, N


def _build_graph(D, H1, H2, A, NG, starts, N):
    """Build + finalize the SPMD Bass graph (identical for all cores)."""
    from concourse import bacc
    import concourse.mybir as mybir
    from concourse.tile import TileContext

    bf = mybir.dt.bfloat16
    f32 = mybir.dt.float32
    KD, K1, K2 = D // 128, H1 // 128, H2 // 128
    M1, M2 = H1 // 128, H2 // 128
    assert D % 128 == 0 and H1 % 128 == 0 and H2 % 128 == 0 and A <= 128

    chunks = []
    c0 = 0
    while c0 < N:
        cw = min(512, N - c0)
        chunks.append((c0, cw))
        c0 += cw

    nc = bacc.Bacc("TRN2")
    st_ext = nc.declare_dram_parameter("st", [KD, 128, N], bf, isOutput=False)
    w1_ext = nc.declare_dram_parameter("w1", [KD, 128, H1], bf, isOutput=False)
    w2_ext = nc.declare_dram_parameter("w2", [K1, 128, H2], bf, isOutput=False)
    w3_ext = nc.declare_dram_parameter("w3", [NG, 128, K2 * A], bf, isOutput=False)
    b1_ext = nc.declare_dram_parameter("b1s", [128, M1], f32, isOutput=False)
    b2_ext = nc.declare_dram_parameter("b2s", [128, M2], f32, isOutput=False)
    b3_ext = nc.declare_dram_parameter("b3s", [A, NG], f32, isOutput=False)
    out_ext = nc.declare_dram_parameter("out", [A, N], f32, isOutput=True)

    add = mybir.AluOpType.add
    amax = mybir.AluOpType.max
    Tanh = mybir.ActivationFunctionType.Tanh

    with TileContext(nc) as tc:
        with (
            tc.tile_pool(name="weights", bufs=1) as wp,
            tc.tile_pool(name="acts", bufs=1) as ap,
            tc.tile_pool(name="psum", bufs=4, space="PSUM") as pp,
            tc.tile_pool(name="psum3", bufs=2, space="PSUM") as pp3,
        ):
            st = [ap.tile([128, N], bf, name=f"st{k}", tag=f"st{k}") for k in range(KD)]
            w1 = [wp.tile([128, H1], bf, name=f"w1_{k}", tag=f"w1_{k}") for k in range(KD)]
            w2 = [wp.tile([128, H2], bf, name=f"w2_{k}", tag=f"w2_{k}") for k in range(K1)]
            w3 = [wp.tile([128, K2 * A], bf, name=f"w3_{j}", tag=f"w3_{j}") for j in range(NG)]
            b1t = wp.tile([128, M1], f32, name="b1t", tag="b1t")
            b2t = wp.tile([128, M2], f32, name="b2t", tag="b2t")
            b3t = wp.tile([A, NG], f32, name="b3t", tag="b3t")
            h1 = [ap.tile([128, N], bf, name=f"h1_{m}", tag=f"h1_{m}") for m in range(M1)]
            hf = [ap.tile([128, N], bf, name=f"hf_{m}", tag=f"hf_{m}") for m in range(M2)]
            osb = ap.tile([A, N], f32, name="osb", tag="osb")

            # Two HWDGE queues: nc.sync (qSP) and nc.scalar (qACT).  Order by
            # first use: chunk-0 state + w1 are critical (first matmuls);
            # w2/w3 stream on the second queue and arrive before layer 2/3.
            c0w = chunks[0][1]
            nc.sync.dma_start(b1t[:], b1_ext[:])
            nc.scalar.dma_start(b2t[:], b2_ext[:])
            nc.scalar.dma_start(b3t[:], b3_ext[:])
            for k in range(KD):
                nc.sync.dma_start(st[k][:, 0:c0w], st_ext[k][:, 0:c0w])
            for k in range(KD):
                eng = nc.sync if k < KD // 2 else nc.scalar
                eng.dma_start(w1[k][:], w1_ext[k])
            for k in range(K1):
                nc.scalar.dma_start(w2[k][:], w2_ext[k])
            for c0_, cw_ in chunks[1:]:
                for k in range(KD):
                    nc.sync.dma_start(
                        st[k][:, c0_ : c0_ + cw_], st_ext[k][:, c0_ : c0_ + cw_]
                    )
            for j in range(NG):
                nc.scalar.dma_start(w3[j][:], w3_ext[j])

            # L3 group j is ready once chunks covering [starts[j], starts[j+1])
            # are done; emit it eagerly after that chunk for scheduling overlap.
            done_j = 0

            for ci, (c0, cw) in enumerate(chunks):
                sl = slice(c0, c0 + cw)
                # L1: h1[m][:, sl] = relu(W1[:, m-tile].T @ st[:, sl] + b1)
                for m in range(M1):
                    ps = pp.tile([128, cw], f32, name="ps", tag="ps")
                    for k in range(KD):
                        nc.tensor.matmul(
                            ps[:],
                            w1[k][:, m * 128 : (m + 1) * 128],
                            st[k][:, sl],
                            start=(k == 0),
                            stop=(k == KD - 1),
                        )
                    nc.vector.tensor_scalar(
                        h1[m][:, sl], ps[:], b1t[:, m : m + 1], 0.0, add, amax
                    )
                # L2
                for m in range(M2):
                    ps = pp.tile([128, cw], f32, name="ps", tag="ps")
                    for k in range(K1):
                        nc.tensor.matmul(
                            ps[:],
                            w2[k][:, m * 128 : (m + 1) * 128],
                            h1[k][:, sl],
                            start=(k == 0),
                            stop=(k == K1 - 1),
                        )
                    nc.vector.tensor_scalar(
                        hf[m][:, sl], ps[:], b2t[:, m : m + 1], 0.0, add, amax
                    )
                # L3 for fully-covered groups
                lim = c0 + cw
                while done_j < NG and starts[done_j + 1] <= lim:
                    j = done_j
                    sj, ej = int(starts[j]), int(starts[j + 1])
                    mj = ej - sj
                    ps = pp3.tile([A, mj], f32, name="ps3", tag="ps3")
                    for k in range(K2):
                        nc.tensor.matmul(
                            ps[:],
                            w3[j][:, k * A : (k + 1) * A],
                            hf[k][:, sj:ej],
                            start=(k == 0),
                            stop=(k == K2 - 1),
                        )
                    nc.scalar.activation(
                        osb[:, sj:ej], ps[:], Tanh, bias=b3t[:, j : j + 1]
                    )
                    nc.sync.dma_start(out_ext[:, sj:ej], osb[:, sj:ej])
                    done_j += 1
            assert done_j == NG

    nc.finalize()
    return nc


def _prepare(state, idx, W1, b1, W2, b2, W3, b3):
    state = np.ascontiguousarray(np.asarray(state, dtype=np.float32))
    idx = np.asarray(idx)
    W1 = np.asarray(W1, dtype=np.float32)
    b1 = np.asarray(b1, dtype=np.float32)
    W2 = np.asarray(W2, dtype=np.float32)
    b2 = np.asarray(b2, dtype=np.float32)
    W3 = np.asarray(W3, dtype=np.float32)
    b3 = np.asarray(b3, dtype=np.float32)

    B, D = state.shape
    G, _, H1 = W1.shape
    H2 = W2.shape[1]
    A = W3.shape[2]
    KD, K1, K2 = D // 128, H1 // 128, H2 // 128

    perm, core_groups, M, starts, N = _make_plan(idx, G)
    NG = len(M)

    key = (D, H1, H2, A, NG, tuple(int(x) for x in starts), N)
    if key not in _graph_cache:
        _graph_cache[key] = _build_graph(D, H1, H2, A, NG, starts, N)
    nc = _graph_cache[key]

    w2_h = np.ascontiguousarray(W2.astype(_BF16).reshape(K1, 128, H2))
    b2_h = np.ascontiguousarray(b2.reshape(K1, 128).T.astype(np.float32))

    in_maps = []
    scatters = []  # per core: list of (sorted_positions, col_start)
    for c in range(G):
        sT = np.zeros((D, N), dtype=_BF16)
        w3_h = np.zeros((NG, 128, K2 * A), dtype=_BF16)
        b3_h = np.zeros((A, NG), dtype=np.float32)
        sc = []
        for j, (head, pos) in enumerate(core_groups[c]):
            s0 = int(starts[j])
            if len(pos):
                sT[:, s0 : s0 + len(pos)] = state[perm[pos]].T.astype(_BF16)
                sc.append((pos, s0))
            # [ki, ko, a] layout so each SBUF k-slice [:, k*A:(k+1)*A] is [128, A]
            w3_h[j] = (
                W3[head].astype(_BF16).reshape(K2, 128, A).transpose(1, 0, 2).reshape(128, K2 * A)
            )
            b3_h[:, j] = b3[head]
        in_maps.append(
            {
                "st": np.ascontiguousarray(sT.reshape(KD, 128, N)),
                "w1": np.ascontiguousarray(W1[c].astype(_BF16).reshape(KD, 128, H1)),
                "w2": w2_h,
                "w3": np.ascontiguousarray(w3_h),
                "b1s": np.ascontiguousarray(b1[c].reshape(K1, 128).T.astype(np.float32)),
                "b2s": b2_h,
                "b3s": b3_h,
            }
        )
        scatters.append(sc)
    return nc, in_maps, scatters, (B, A)


def _run(state, idx, W1, b1, W2, b2, W3, b3, trace=False, trace_kwargs=None):
    from concourse.bass_utils import run_bass_kernel_spmd

    nc, in_maps, scatters, (B, A) = _prepare(state, idx, W1, b1, W2, b2, W3, b3)
    res = run_bass_kernel_spmd(
        nc,
        in_maps,
        core_ids=list(range(_NCORES)),
        trace=trace,
        **(trace_kwargs or {}),
    )
    out = np.zeros((B, A), dtype=np.float32)
    for c in range(len(scatters)):
        o = np.asarray(res.results[c]["out"], dtype=np.float32)  # [A, N]
        for pos, s0 in scatters[c]:
            out[pos] = o[:, s0 : s0 + len(pos)].T
    return out, res


def kernel(**inputs) -> np.ndarray:
    out, _ = _run(**inputs)
    return out
